# revision 10
# baseline (speedup 1.0000x reference)
"""Correspondence Soft-NMS on 8 Trainium2 NeuronCores (Bass/Tile), v3.

Math: penalty_i = sum_j [s_j > s_i] * exp(-(d2src_ij + d2tgt_ij)/delta^2)
      out_i    = s_i * exp(-penalty_i / sigma)

Design:
  * Host sorts by score desc -> suppressors are the positional prefix (ties
    fixed by a host-side correction).
  * Pairwise dots are PRE-SCALED into the Schraudolph integer domain:
    psum x = A_SC*(sq_j - 2 P_i.P_j), A_SC = -100*log2(e)*2^23. With
    dp_i = A_SC*sq_i + (127*2^23 - C), rne(max(x+dp,0)) bitcast to f32
    IS ~exp(z) (exact 0 for far pairs).
  * bf16 limbs, 6 limb-pair groups (00,11,02,10,01,20) -> K=42, duplicated
    at partitions 0/64 so consecutive matmuls alternate PE row-groups.
  * PSUM is one [128,4096] ring (8 banks). Matmuls fill 1024-col windows
    round-robin; consumers chase them with range-level deps.
  * Two consumer streams, globally balanced:
      ACT: exp(SC_ACT*x + bias_i) + fused row-sum over 1-3 window spans.
      DVE: pass1 i32 = rne(max(x+dp,0)) (tensor_scalar), or for boundary
           windows rne(max(x+dp, M)) (scalar_tensor_tensor, M=2^30 on
           excluded cols -> bitcast 2.0 each, corrected in the final exp
           bias); pass2 sums the bitcast with accum_out.
  * No mask matmuls at all -> PE runs a clean 2-matmul-per-window stream.
"""

import sys
import types

import numpy as np
import ml_dtypes


def _ensure_axon_hooks():
    try:
        import antenv.axon_hooks  # noqa: F401
        return
    except ImportError:
        pass
    try:
        import antenv
    except ImportError:
        return
    mod = types.ModuleType("antenv.axon_hooks")
    mod._hook = None

    def set_axon_ntff_profile_hook(h):
        mod._hook = h

    def get_axon_ntff_profile_hook():
        return mod._hook

    mod.set_axon_ntff_profile_hook = set_axon_ntff_profile_hook
    mod.get_axon_ntff_profile_hook = get_axon_ntff_profile_hook
    sys.modules["antenv.axon_hooks"] = mod
    antenv.axon_hooks = mod


_ensure_axon_hooks()

import concourse.bass as bass
import concourse.bacc as bacc
import concourse.tile as tile
import concourse.mybir as mybir
import concourse.bass_utils as bass_utils

N = 8192
NCORES = 8
P = 128
SLOTS = 8
W = 1024          # window width (cols)
RING = 4          # ring capacity in windows (4*1024 = 8 psum banks)
KG = 42           # 6 limb-pair groups x 7
DELTA = 0.1
SIGMA = 0.05
FINAL_SCALE = -1.0 / SIGMA           # -20.0

A_SC = -100.0 * np.log2(np.e) * 2.0**23
ALPHA = np.float32(A_SC / 2.0**15)
A_SC_EFF = float(ALPHA) * 2.0**15
SC_ACT = float(np.float32(-100.0 / A_SC_EFF))
C_CAL = 460000.0
OFF = 127.0 * 2.0**23 - C_CAL
MBIG = 2.0**30    # stt mask value; bitcasts to 2.0f per excluded element

BF16 = mybir.dt.bfloat16
F32 = mybir.dt.float32
I32 = mybir.dt.int32
NPBF16 = ml_dtypes.bfloat16

_cache = {}

# windows assigned to the DVE stream beyond the (mandatory) boundary ones
_DVE_EXTRA = {(8, 0), (7, 0)}


def _schedule():
    """Static window walk + consumer spans.

    Returns (seq, spans, npart, slot_parts):
      seq:   [(slot, w, ring_off)] in issue order (slot-major)
      spans: [dict(kind, slot, ring_off, ncols, first_w, pcol)]
      slot_parts: {slot: (pcol_start, n_pcols)} contiguous partial columns
    """
    seq = []
    k = 0
    for s in range(1, SLOTS + 1):
        for w in range(s):
            seq.append((s, w, (k % RING) * W))
            k += 1

    def stream_of(s, w):
        if w == s - 1:
            return "D"
        return "D" if (s, w) in _DVE_EXTRA else "A"

    spans = []
    cur = None
    for k, (s, w, off) in enumerate(seq):
        st = stream_of(s, w)
        if (cur is not None and cur["kind"] == st and cur["slot"] == s
                and cur["ring_off"] + cur["ncols"] == off
                and cur["nw"] < (3 if st == "A" else 2)):
            cur["ncols"] += W
            cur["nw"] += 1
            cur["end_k"] = k
        else:
            if cur is not None:
                spans.append(cur)
            cur = dict(kind=st, slot=s, ring_off=off, ncols=W, first_w=w,
                       nw=1, end_k=k)
    spans.append(cur)

    # partial columns: slot-major contiguous
    slot_parts = {}
    pc = 0
    for s in range(1, SLOTS + 1):
        s_spans = [sp for sp in spans if sp["slot"] == s]
        slot_parts[s] = (pc, len(s_spans))
        for sp in s_spans:
            sp["pcol"] = pc
            pc += 1
    return seq, spans, pc, slot_parts


def _build_body(tc, d):
    nc = tc.nc
    seq, spans, npart, slot_parts = _schedule()

    with tc.tile_pool(name="const", bufs=1) as cpool, \
         tc.tile_pool(name="psum", bufs=1, space="PSUM") as ppool:

        bundle_sb = cpool.tile([P, SLOTS * P], BF16, tag="bundle")
        lhsT_sb = bundle_sb[:, 0: SLOTS * P]
        m_sb = cpool.tile([P, W], BF16, tag="mmask")        # 2^30 on excluded
        rhs_sb = cpool.tile([P, N], BF16, tag="rhs")
        fb_sb = cpool.tile([P, 3 * SLOTS + 2], F32, tag="fbundle")
        dp_sb = fb_sb[:, 0:SLOTS]
        abias_sb = fb_sb[:, SLOTS: 2 * SLOTS]
        srow_sb = fb_sb[:, 2 * SLOTS: 3 * SLOTS]
        bias2_sb = fb_sb[:, 3 * SLOTS: 3 * SLOTS + 1]
        warm_sb = fb_sb[:, 3 * SLOTS + 1: 3 * SLOTS + 2]
        partials = cpool.tile([P, npart], F32, tag="partials")
        penalty = cpool.tile([P, SLOTS], F32, tag="penalty")
        decay = cpool.tile([P, SLOTS], F32, tag="decay")
        out_sb = cpool.tile([P, SLOTS], F32, tag="outsb")
        i32_t = cpool.tile([P, 2 * W], I32, tag="i32t")
        junk_a = cpool.tile([P, 3 * W], F32, tag="junka")
        junk_d = cpool.tile([P, 2 * W], F32, tag="junkd")
        warmj = cpool.tile([P, 1], F32, tag="warmj")

        ring = ppool.tile([P, RING * W], F32, tag="ring")

        # --- DMA: two HWDGE queues (SP + ACT) issue in parallel.
        # First-needed pieces are smallest so matmul 0 starts early.
        rhs_ap = d["rhs"].ap()
        bnd_ap = d["bundle"].ap()
        nc.sync.dma_start(rhs_sb[0:KG, 0:512], rhs_ap[:, 0:512])
        nc.sync.dma_start(rhs_sb[64:64 + KG, 0:512], rhs_ap[:, 0:512])
        nc.sync.dma_start(m_sb[:], d["mmask"].ap())
        nc.scalar.dma_start(fb_sb[:], d["fbundle"].ap())
        nc.scalar.dma_start(bundle_sb[:, 0:P], bnd_ap[:, 0:P])
        nc.scalar.dma_start(bundle_sb[:, P:], bnd_ap[:, P:])
        for (c0, c1) in ((512, 2048), (2048, 5120), (5120, 8192)):
            nc.sync.dma_start(rhs_sb[0:KG, c0:c1], rhs_ap[:, c0:c1])
            nc.scalar.dma_start(rhs_sb[64:64 + KG, c0:c1], rhs_ap[:, c0:c1])
        nc.scalar.activation(
            warmj[:], warm_sb[:], mybir.ActivationFunctionType.Exp,
            scale=1.0,
        )

        # --- PE warm-up: dependency-free matmuls on memset SBUF keep the
        # PE busy ~3.6us during the input DMA so the HAM clock gate opens
        # (1.2 -> 2.4 GHz) before real work begins. Dead writes: every ring
        # slot is later overwritten by a start=True matmul.
        nc.vector.memset(i32_t[:, 0:512], 0)
        wsrc = i32_t[:, 0:512].bitcast(BF16)  # [128, 1024] zeros
        for k in range(6):
            base = 64 * (k % 2)
            nc.tensor.matmul(
                ring[:, 512 * k: 512 * (k + 1)],
                lhsT=wsrc[base: base + KG, 0:P],
                rhs=wsrc[base: base + KG, 0:512],
                start=True,
                stop=True,
                tile_position=(base, 0),
            )

        # --- pipeline: matmuls + consumers in ring order ---
        spans_by_end = {}
        for sp in spans:
            spans_by_end.setdefault(sp["end_k"], []).append(sp)

        # DVE pass2 is deferred one DVE-span: pass1(j+1) runs before
        # pass2(j) so the ring slot frees without waiting for the sum.
        dve_idx = [0]
        pending = []

        def emit_pass2(sp):
            nco = sp["ncols"]
            half = sp["dbuf"] * W
            nc.vector.tensor_scalar(
                junk_d[:, 0:nco],
                i32_t[:, half: half + nco].bitcast(F32), 0.0, None,
                op0=mybir.AluOpType.add, op1=mybir.AluOpType.add,
                accum_out=partials[:, sp["pcol"]: sp["pcol"] + 1],
            )

        def emit_span(sp):
            s = sp["slot"]
            off = sp["ring_off"]
            nco = sp["ncols"]
            pc = sp["pcol"]
            src = ring[:, off: off + nco]
            if sp["kind"] == "A":
                nc.scalar.activation(
                    junk_a[:, 0:nco], src,
                    mybir.ActivationFunctionType.Exp,
                    bias=abias_sb[:, s - 1: s],
                    scale=SC_ACT,
                    accum_out=partials[:, pc: pc + 1],
                )
                return
            sp["dbuf"] = dve_idx[0] % 2
            dve_idx[0] += 1
            half = sp["dbuf"] * W
            boundary = (sp["first_w"] + sp["nw"] - 1 == s - 1)
            if boundary:
                assert sp["nw"] == 1, "boundary spans are single-window"
                nc.vector.scalar_tensor_tensor(
                    i32_t[:, half: half + nco], src, dp_sb[:, s - 1: s],
                    m_sb[:],
                    op0=mybir.AluOpType.add, op1=mybir.AluOpType.max,
                )
            else:
                nc.vector.tensor_scalar(
                    i32_t[:, half: half + nco], src, dp_sb[:, s - 1: s], 0.0,
                    op0=mybir.AluOpType.add, op1=mybir.AluOpType.max,
                )
            while pending:
                emit_pass2(pending.pop(0))
            pending.append(sp)

        mm = 0
        for k, (s, w, off) in enumerate(seq):
            for h in range(2):
                base = 64 * (mm % 2)
                mm += 1
                nc.tensor.matmul(
                    ring[:, off + 512 * h: off + 512 * h + 512],
                    lhsT=lhsT_sb[base: base + KG, bass.ts(s - 1, P)],
                    rhs=rhs_sb[base: base + KG,
                               w * W + 512 * h: w * W + 512 * h + 512],
                    start=True,
                    stop=True,
                    tile_position=(base, 0),
                )
            for sp in spans_by_end.get(k, ()):
                emit_span(sp)
        while pending:
            emit_pass2(pending.pop(0))

        for s in range(1, SLOTS + 1):
            p0, np_ = slot_parts[s]
            nc.vector.tensor_reduce(
                penalty[:, s - 1: s], partials[:, p0: p0 + np_],
                axis=mybir.AxisListType.X, op=mybir.AluOpType.add,
            )

        # decay = exp(-20*penalty + 40*n_excl) ; bias2 folds out the 2.0s
        # contributed by each excluded boundary element.
        nc.scalar.activation(
            decay[:], penalty[:], mybir.ActivationFunctionType.Exp,
            bias=bias2_sb[:, 0:1], scale=FINAL_SCALE,
        )
        nc.vector.tensor_mul(out_sb[:], decay[:], srow_sb[:])
        nc.sync.dma_start(d["out"].ap(), out_sb[:])


def _build():
    if "nc" in _cache:
        return _cache["nc"]
    nc = bacc.Bacc(
        "TRN2",
        target_bir_lowering=False,
        debug=False,
        enable_asserts=False,
    )
    d = {
        "bundle": nc.dram_tensor(
            "bundle", [P, SLOTS * P], BF16, kind="ExternalInput"
        ),
        "mmask": nc.dram_tensor("mmask", [P, W], BF16, kind="ExternalInput"),
        "rhs": nc.dram_tensor("rhs", [KG, N], BF16, kind="ExternalInput"),
        "fbundle": nc.dram_tensor(
            "fbundle", [P, 3 * SLOTS + 2], F32, kind="ExternalInput"
        ),
        "out": nc.dram_tensor("out", [P, SLOTS], F32, kind="ExternalOutput"),
    }
    with tile.TileContext(nc) as tc:
        _build_body(tc, d)
    nc.compile()
    _cache["nc"] = nc
    return nc


def _split3(x64):
    a0 = x64.astype(NPBF16)
    r = x64 - a0.astype(np.float64)
    a1 = r.astype(NPBF16)
    r2 = r - a1.astype(np.float64)
    a2 = r2.astype(NPBF16)
    return a0, a1, a2


def _prepare_inputs(src_points, tgt_points, scores):
    scores = np.asarray(scores, np.float32)
    src = np.asarray(src_points, np.float32)
    tgt = np.asarray(tgt_points, np.float32)

    order = np.argsort(-scores.astype(np.float64), kind="stable")
    s_sorted = scores[order]
    P6 = np.concatenate([src, tgt], axis=1).astype(np.float64)[order]
    sq = np.sum(P6 * P6, axis=1)

    A7 = np.concatenate([(-2.0 * P6).T, np.ones((1, N))], axis=0) * float(ALPHA)
    B7 = np.concatenate([P6.T, sq[None, :]], axis=0) * 2.0**15
    A0, A1, A2 = _split3(A7)
    B0, B1, B2 = _split3(B7)
    lhsT_full = np.concatenate([A0, A1, A0, A1, A0, A2], axis=0)  # [42,N]
    rhs42 = np.ascontiguousarray(
        np.concatenate([B0, B1, B2, B0, B1, B0], axis=0))

    dp_full = (A_SC_EFF * sq + OFF).astype(np.float32)
    abias_full = (-100.0 * sq).astype(np.float32)

    in_maps = []
    for c in range(NCORES):
        gs = 8 * np.arange(SLOTS) + c
        rows = (gs[:, None] * P + np.arange(P)[None, :]).reshape(-1)
        lhsT_c = np.zeros((P, SLOTS * P), NPBF16)
        lhsT_c[0:KG] = lhsT_full[:, rows]
        lhsT_c[64:64 + KG] = lhsT_full[:, rows]
        dp_c = dp_full[rows].reshape(SLOTS, P).T
        abias_c = abias_full[rows].reshape(SLOTS, P).T
        srow_c = s_sorted[rows].reshape(SLOTS, P).T.astype(np.float32)
        f = np.arange(W)[None, :]
        p = np.arange(P)[:, None]
        m_c = (MBIG * (f >= (P * c + p))).astype(NPBF16)
        # n excluded per row (same for every slot) -> decay bias +40*n
        n_excl = (W - P * c - np.arange(P)).astype(np.float64)
        bias2_c = (40.0 * n_excl).astype(np.float32).reshape(P, 1)
        warm_c = np.zeros((P, 1), np.float32)
        fbundle_c = np.ascontiguousarray(np.concatenate(
            [dp_c, abias_c, srow_c, bias2_c, warm_c], axis=1
        ).astype(np.float32))
        in_maps.append({
            "bundle": np.ascontiguousarray(lhsT_c),
            "mmask": np.ascontiguousarray(m_c),
            "rhs": rhs42,
            "fbundle": fbundle_c,
        })
    return in_maps, order, s_sorted, P6


def _tie_correction(out_sorted, s_sorted, P6):
    ties = np.flatnonzero(np.diff(s_sorted) == 0.0)
    if ties.size == 0:
        return out_sorted
    out = out_sorted.copy()
    runs = []
    start = ties[0]
    prev = ties[0]
    for t in ties[1:]:
        if t != prev + 1:
            runs.append((start, prev + 1))
            start = t
        prev = t
    runs.append((start, prev + 1))
    for (a, b) in runs:
        idx = np.arange(a, b + 1)
        for ii in range(1, idx.size):
            i = idx[ii]
            js = idx[:ii]
            d2 = np.sum((P6[i] - P6[js]) ** 2, axis=1)
            corr = np.sum(np.exp(-100.0 * d2))
            out[i] = out[i] * np.exp(-FINAL_SCALE * corr)
    return out


LAST_EXEC_TIME_NS = None


def kernel(src_points, tgt_points, scores):
    global LAST_EXEC_TIME_NS
    nc = _build()
    in_maps, order, s_sorted, P6 = _prepare_inputs(src_points, tgt_points, scores)
    res = bass_utils.run_bass_kernel_spmd(nc, in_maps, core_ids=list(range(NCORES)))
    LAST_EXEC_TIME_NS = res.exec_time_ns

    out_sorted = np.empty((N // P, P), np.float32)
    for c in range(NCORES):
        gs = 8 * np.arange(SLOTS) + c
        out_sorted[gs, :] = np.asarray(res.results[c]["out"]).T  # [8,128]
    out_sorted = out_sorted.reshape(N)
    out_sorted = _tie_correction(out_sorted, s_sorted, P6)

    out = np.empty(N, np.float32)
    out[order] = out_sorted
    return out
